# revision 18
# baseline (speedup 1.0000x reference)
"""Trainium2 Bass kernel for a dense transformer block (pre-LN, causal MHA + 4x MLP).

v2: bf16 datapath + sharded LN1 stats + column-strip attention with pipelined
AllToAll.

Sharding over 8 NeuronCores: attention is head-sharded 8 ways (each core does
H/8=2 heads for BOTH batches), then AllToAll re-shards activations to
(batch, token-block) shards so the out-projection and MLP run row-sharded with
full weights.

Key structure vs v1:
- All matmul operands are bf16 (PSUM accumulation stays fp32): same PE
  throughput as f32r but no >=256 free-size restriction, half the DMA bytes,
  half the collective bytes, 2x DVE on sbuf-only ops.
- LN1 statistics are computed for OWN 512 tokens only (per-core shard) and
  AllGathered (3x512 bf16 rows: -mu, std, 1/std) while x streams in; the
  mean/bias corrections enter each projection as one K=2 rank-2 matmul.
- Attention runs in 128-wide q column strips across all q-blocks, so the
  AllToAll fires in 3 pieces (widths 256/128/128 of every destination block)
  pipelined behind the remaining strips; out-projection consumes each piece
  as it lands, filling PE gaps in the ACT(exp)-bound attention phase.
- exp is batched over up-to-8 key chunks per ACT instruction; the causal mask
  needs exactly one 128x128 affine_select per (head, q-strip).
- LN2's ACT chain (Ln/Exp) is deferred until all strips are in, so the
  activation table never thrashes: Ln/Exp load twice, Gelu once.
"""
import numpy as np
from contextlib import ExitStack

import concourse.bass as bass
import concourse.mybir as mybir
import concourse.tile as tile
from concourse import bacc

F32 = mybir.dt.float32
BF16 = mybir.dt.bfloat16
AF = mybir.ActivationFunctionType
ALU = mybir.AluOpType


class Cfg:
    def __init__(self, D=1024, DFF=4096, H=16, T=2048, B=2, TP=4, HD=64):
        self.D, self.DFF, self.H, self.T, self.B, self.TP, self.HD = D, DFF, H, T, B, TP, HD
        self.NC = B * TP              # cores
        self.KD = D // 128            # feature chunks
        self.FD = DFF // 128          # hidden chunks
        self.LCH = 512                # token chunk (stats shard + stream)
        self.NCH = B * T // self.LCH  # flat chunks (== NC)
        self.NTL = T // self.LCH      # chunks per batch
        self.NHC = H // self.NC       # heads per core
        self.HC = self.NHC * HD       # head feature columns per core
        self.TQ = T // TP             # owned tokens per core
        self.NKC = T // 128           # key chunks per batch
        self.NSUB = T // 128          # 128-wide q substrips per batch
        self.QB = T // 512            # q blocks per batch
        self.PIECES = [[0, 1, 2], [3]]   # a2a pieces over strip idx i in 0..3
        self.GRP = 8                  # exp batch: key chunks per ACT op
        self.OCB = 256
        self.HCB = 512
        self.KHB = 8
        assert self.NCH == self.NC and self.HC == 128


CFG = Cfg()


def emit(ctx: ExitStack, tc: tile.TileContext, io: dict, cfg: Cfg):
    nc = tc.nc
    c = cfg
    rearr = lambda ap: ap.rearrange("(o p) t -> p o t", p=128)

    def mm(ps, lhsT, rhs, start, stop):
        nc.tensor.matmul(ps, lhsT, rhs, start=start, stop=stop)

    # ---------------- constant / persistent pools ----------------
    const = ctx.enter_context(tc.tile_pool(name="const", bufs=1))
    small = ctx.enter_context(tc.tile_pool(name="small", bufs=1))
    bcast = ctx.enter_context(tc.tile_pool(name="bcast", bufs=2))
    dram = ctx.enter_context(tc.tile_pool(name="dram", bufs=1, space="DRAM"))
    wgt = ctx.enter_context(tc.tile_pool(name="wgt", bufs=1))

    from concourse.masks import make_identity
    ident = const.tile([128, 128], BF16, tag="ident")
    make_identity(nc, ident[:])
    ones = const.tile([128, 1], BF16, tag="ones")
    nc.gpsimd.memset(ones[:], 1.0)
    ones_f = const.tile([128, 1], F32, tag="ones_f")
    nc.gpsimd.memset(ones_f[:], 1.0)
    eps_t = const.tile([1, 1], F32, tag="eps")
    nc.gpsimd.memset(eps_t[:], 1e-5)

    # small parameter tiles (fp32 biases packed per-feature)
    pm = const.tile([128, 2 * c.KD + c.FD], F32, tag="pm", name="pm")
    nc.gpsimd.dma_start(pm[:], io["pm"][:])
    bo = pm[:, 0:c.KD]
    bf1 = pm[:, c.KD:c.KD + c.FD]
    bf2 = pm[:, c.KD + c.FD:]
    # rank-2 LN fixups: rows [wsum; bias] per projection, bf16
    pv2 = const.tile([2, 3 * c.HC], BF16, tag="pv2", name="pv2")
    nc.gpsimd.dma_start(pv2[:], io["pv2"][:])
    pvq, pvk, pvv = (pv2[:, i * c.HC:(i + 1) * c.HC] for i in range(3))

    # weights: qkv slices + full wo resident; first fc chunks prefetched on
    # the gpsimd queue so the fc pipeline starts hot
    wq = wgt.tile([128, c.KD, c.HC], BF16, tag="wq")
    nc.gpsimd.dma_start(wq[:], rearr(io["wq"]))
    wk = wgt.tile([128, c.KD, c.HC], BF16, tag="wk")
    nc.gpsimd.dma_start(wk[:], rearr(io["wk"]))
    wv = wgt.tile([128, c.KD, c.HC], BF16, tag="wv")
    nc.gpsimd.dma_start(wv[:], rearr(io["wv"]))
    wo = wgt.tile([128, c.KD, c.D], BF16, tag="wo")
    wf1_0 = wgt.tile([128, c.KD, c.HCB], BF16, tag="wf1_0")
    wf2r = io["wf2"].rearrange("(o p) d -> p o d", p=128)
    wf2_0 = wgt.tile([128, c.KHB, c.OCB], BF16, tag="wf2_0")

    # ---------------- phase 0: own-chunk LN1 stats + AllGather -------------
    ag_in = dram.tile([3, c.LCH], BF16, tag="ag_in", name="ag_in")
    ag_out = dram.tile([3 * c.NC, c.LCH], BF16, tag="ag_out", name="ag_out")

    xo = const.tile([128, c.KD, c.LCH], BF16, tag="xo", name="xo")
    nc.sync.dma_start(xo[:], rearr(io["xo"]))

    inv_d = 1.0 / c.D
    with ExitStack() as ph0:
        ph0.enter_context(tc.high_priority())
        xsqp = ph0.enter_context(tc.tile_pool(name="xsqp", bufs=2))
        pmm = ph0.enter_context(tc.tile_pool(name="pmm0", bufs=2, space="PSUM"))
        ps1 = pmm.tile([128, c.LCH], F32, tag="mm", name="ps1")[0:1, :]
        for o in range(c.KD):
            mm(ps1, ones[:], xo[:, o, :], o == 0, o == c.KD - 1)
        ps2 = pmm.tile([128, c.LCH], F32, tag="mm", name="ps2")[0:1, :]
        for o in range(c.KD):
            xsq = xsqp.tile([128, c.LCH], BF16, tag="xsq", name="xsq")
            nc.vector.tensor_tensor(xsq[:], xo[:, o, :], xo[:, o, :], ALU.mult)
            mm(ps2, ones[:], xsq[:], o == 0, o == c.KD - 1)
        mu = small.tile([1, c.LCH], F32, tag="mu", name="mu")
        nc.vector.tensor_scalar_mul(mu, ps1, inv_d)
        ex2 = small.tile([1, c.LCH], F32, tag="ex2", name="ex2")
        nc.vector.tensor_scalar_mul(ex2, ps2, inv_d)
        var = small.tile([1, c.LCH], F32, tag="var", name="var")
        nc.vector.tensor_tensor(var, mu, mu, ALU.mult)
        nc.vector.tensor_tensor(var, ex2, var, ALU.subtract)
        lnv = small.tile([1, c.LCH], F32, tag="lnv", name="lnv")
        nc.scalar.activation(lnv, var, AF.Ln, bias=eps_t[:])
        # compute-engine APs must start at partition 0/32/64: keep each stat
        # row in its own tile and let the DMA place them in ag_in rows
        negmu_t = small.tile([1, c.LCH], BF16, tag="negmu_t", name="negmu_t")
        nc.vector.tensor_scalar_mul(negmu_t, mu, -1.0)
        std_t = small.tile([1, c.LCH], BF16, tag="std_t", name="std_t")
        nc.scalar.activation(std_t, lnv, AF.Exp, scale=0.5)
        A_t = small.tile([1, c.LCH], BF16, tag="A_t", name="A_t")
        nc.scalar.activation(A_t, lnv, AF.Exp, scale=-0.5)
        nc.scalar.dma_start(ag_in[0:1, :], negmu_t[:])
        nc.scalar.dma_start(ag_in[1:2, :], std_t[:])
        nc.scalar.dma_start(ag_in[2:3, :], A_t[:])
        nc.gpsimd.collective_compute(
            "AllGather", ALU.bypass, replica_groups=[list(range(c.NC))],
            ins=[ag_in[:].opt()], outs=[ag_out[:].opt()])
    # land [-mu; std] rows at partitions 0-1 and A rows at partition 0 by
    # de-interleaving the AllGather result in the DMA
    ago3 = ag_out[:].rearrange("(ch s) t -> s ch t", s=3)
    nmstd_sb = const.tile([2, c.NCH, c.LCH], BF16, tag="nmstd", name="nmstd")
    nc.scalar.dma_start(nmstd_sb[:], ago3[0:2, :, :])
    A_sb = const.tile([1, c.NCH, c.LCH], BF16, tag="A_sb", name="A_sb")
    nc.scalar.dma_start(A_sb[:], ago3[2:3, :, :])
    # late-needed weights stream on the scalar queue, idle during QKV
    nc.scalar.dma_start(wo[:], rearr(io["wo"]))
    nc.scalar.dma_start(wf1_0[:], rearr(io["wf1"])[:, :, 0:c.HCB])
    nc.scalar.dma_start(wf2_0[:], wf2r[:, 0:c.KHB, 0:c.OCB])

    isc = 1.0 / float(np.sqrt(c.HD))

    # a2a bounce buffers, one per strip piece, each in its OWN dram pool so
    # the framework's collective/DMA aliasing fences don't serialize pieces
    a2a_in, a2a_out = [], []
    for p, I in enumerate(c.PIECES):
        din = ctx.enter_context(tc.tile_pool(name=f"da{p}", bufs=1, space="DRAM"))
        dout = ctx.enter_context(tc.tile_pool(name=f"db{p}", bufs=1, space="DRAM"))
        a2a_in.append(din.tile([c.NC * c.HC, 128 * len(I)], BF16,
                               tag=f"a2a_in{p}", name=f"a2a_in{p}"))
        a2a_out.append(dout.tile([c.NC * c.HC, 128 * len(I)], BF16,
                                 tag=f"a2a_out{p}", name=f"a2a_out{p}"))

    # persistent attention tiles
    kvqy = ctx.enter_context(tc.tile_pool(name="kvqy", bufs=1))
    kT = [kvqy.tile([128, c.T], BF16, tag=f"kT{b}", name=f"kT{b}")
          for b in range(c.B)]
    qT = [kvqy.tile([128, c.T], BF16, tag=f"qT{b}", name=f"qT{b}")
          for b in range(c.B)]
    v_sb = [[kvqy.tile([128, c.NHC * 65], BF16, tag=f"v{b}_{a}",
                       name=f"v{b}_{a}") for a in range(c.NKC)]
            for b in range(c.B)]
    for b in range(c.B):
        for a in range(c.NKC):
            nc.vector.tensor_copy(
                v_sb[b][a][:].rearrange("p (h e) -> p h e", e=65)[:, :, 64:65],
                ones_f[:, 0:1].to_broadcast((128, c.NHC, 1)))

    # ------- merged phase 1+2: QKV chunks with eager piece-0 attention, ----
    # ------- then post-QKV strips with pipelined A2A + interleaved oproj ---
    x2b = ctx.enter_context(tc.tile_pool(name="x2b", bufs=1)).tile(
        [128, c.KD, c.TQ], BF16, tag="x2b")
    x2n = ctx.enter_context(tc.tile_pool(name="x2n", bufs=1)).tile(
        [128, c.KD, c.TQ], BF16, tag="x2n")
    mu2 = small.tile([1, c.TQ], F32, tag="mu2", name="mu2")
    ex22 = small.tile([1, c.TQ], F32, tag="ex22", name="ex22")

    def substrip(b, h, qb, i, psy, grp, sgrp_p, ssb_p):
        """q columns [128*s, 128*s+128) of batch b, head h."""
        s = 4 * qb + i
        n = s + 1                     # key chunks 0..s, chunk s diagonal
        rs = slice(64 * h, 64 * h + 64)
        qsl = slice(128 * s, 128 * s + 128)
        ngrp = (n + grp - 1) // grp
        for g in range(ngrp):
            a0, a1 = g * grp, min((g + 1) * grp, n)
            gw = (a1 - a0) * 128
            sgrp = sgrp_p.tile([128, grp * 128], F32, tag="s", name="sgrp")
            for a in range(a0, a1):
                r = (a - a0) * 128
                mm(sgrp[:, r:r + 128], kT[b][rs, a * 128:(a + 1) * 128],
                   qT[b][rs, qsl], True, True)
            ssb = ssb_p.tile([128, grp * 128], BF16, tag="ssb", name="ssb")
            nc.scalar.activation(ssb[:, 0:gw], sgrp[:, 0:gw], AF.Exp,
                                 scale=isc)
            if a1 == n:  # group holds the diagonal chunk
                r = (n - 1 - a0) * 128
                nc.gpsimd.affine_select(
                    out=ssb[:, r:r + 128], in_=ssb[:, r:r + 128],
                    compare_op=ALU.is_ge, fill=0.0,
                    base=0, pattern=[[1, 128]], channel_multiplier=-1)
            for a in range(a0, a1):
                r = (a - a0) * 128
                mm(psy, v_sb[b][a][:, h * 65:h * 65 + 65],
                   ssb[:, r:r + 128], a == 0, a == n - 1)

    def quad(b, qb, i, p, i_rel, grp, pools):
        """both heads of q substrip (b, 4qb+i) + normalize + a2a_in DMA."""
        psy_p, ystr_p, rcp_p, rb_p, sgrp_p, ssb_p = pools
        ystr = ystr_p.tile([128, 128], BF16, tag="ystr", name="ystr")
        psy2 = psy_p.tile([65, 256], F32, tag="y", name="psy")
        for h in range(c.NHC):
            substrip(b, h, qb, i, psy2[:, 128 * h:128 * h + 128],
                     grp, sgrp_p, ssb_p)
        rcp = rcp_p.tile([1, 256], F32, tag="rcp", name="rcp")
        nc.vector.reciprocal(rcp, psy2[64:65, :])
        rb = rb_p.tile([64, 256], F32, tag="rb", name="rb")
        nc.gpsimd.partition_broadcast(rb[:], rcp)
        for h in range(c.NHC):
            nc.vector.tensor_tensor(
                ystr[64 * h:64 * h + 64, :],
                psy2[0:64, 128 * h:128 * h + 128],
                rb[:, 128 * h:128 * h + 128], ALU.mult)
        j = b * c.TP + qb
        nc.sync.dma_start(
            a2a_in[p][c.HC * j:c.HC * (j + 1),
                      128 * i_rel:128 * (i_rel + 1)],
            ystr[:])

    def a2a(p):
        nc.gpsimd.collective_compute(
            "AllToAll", ALU.bypass, replica_groups=[list(range(c.NC))],
            ins=[a2a_in[p][:].opt()], outs=[a2a_out[p][:].opt()])

    xT = io["xT"]  # [D, B*T] bf16
    with ExitStack() as ph1:
        xcp = ph1.enter_context(tc.tile_pool(name="xcp", bufs=3))
        vtp = ph1.enter_context(tc.tile_pool(name="vtp", bufs=2))
        ptr = ph1.enter_context(tc.tile_pool(name="ptr", bufs=1, space="PSUM"))
        pmm = ph1.enter_context(tc.tile_pool(name="pmm1", bufs=3, space="PSUM"))
        sgrpE = ph1.enter_context(tc.tile_pool(name="sgrpE", bufs=2, space="PSUM"))
        psyE = ph1.enter_context(tc.tile_pool(name="psyE", bufs=2, space="PSUM"))
        ssbE = ph1.enter_context(tc.tile_pool(name="ssbE", bufs=6))
        rcpE = ph1.enter_context(tc.tile_pool(name="rcpE", bufs=8))
        rbE = ph1.enter_context(tc.tile_pool(name="rbE", bufs=8))
        ystrE = ph1.enter_context(tc.tile_pool(name="ystrE", bufs=12))
        poolsE = (psyE, ystrE, rcpE, rbE, sgrpE, ssbE)
        for ch in range(c.NCH):
            b, t = ch // c.NTL, ch % c.NTL
            tsl = slice(ch * c.LCH, (ch + 1) * c.LCH)
            lsl = slice(t * c.LCH, (t + 1) * c.LCH)
            xc = xcp.tile([128, c.KD, c.LCH], BF16, tag="xc")
            nc.sync.dma_start(xc[:], rearr(xT)[:, :, tsl])
            nm_std = nmstd_sb[:, ch, :]
            Ab = bcast.tile([128, c.LCH], BF16, tag="Ab", name="Ab")
            nc.gpsimd.partition_broadcast(Ab, A_sb[0:1, ch, :])

            for (wt, pvx, dst) in ((wq, pvq, qT), (wk, pvk, kT)):
                ps = pmm.tile([128, c.LCH], F32, tag="mm", name="psqk")
                for o in range(c.KD):
                    mm(ps, wt[:, o, :], xc[:, o, :], o == 0, False)
                mm(ps, pvx, nm_std, False, True)
                nc.vector.tensor_tensor(dst[b][:, lsl], ps, Ab, ALU.mult)

            # v: feature-major then PE-transpose into token-major v_sb
            ps = pmm.tile([128, c.LCH], F32, tag="mm", name="psv")
            for o in range(c.KD):
                mm(ps, wv[:, o, :], xc[:, o, :], o == 0, False)
            mm(ps, pvv, nm_std, False, True)
            vT = vtp.tile([128, c.LCH], BF16, tag="vT", name="vT")
            nc.vector.tensor_tensor(vT[:], ps, Ab, ALU.mult)
            for s2 in range(c.LCH // 128):
                a = t * (c.LCH // 128) + s2
                pst = ptr.tile([128, 128], BF16, tag="tr", name="pst")
                nc.tensor.transpose(pst[:], vT[:, s2 * 128:(s2 + 1) * 128],
                                    ident[:])
                v3 = v_sb[b][a][:].rearrange("p (h e) -> p h e", e=65)[:, :, 0:64]
                p3 = pst[:].rearrange("p (h e) -> p h e", e=64)
                nc.vector.tensor_copy(v3, p3)

            # eager piece-0 attention for this chunk (i = 0, 1)
            for i_rel, i in enumerate(c.PIECES[0]):
                quad(b, t, i, 0, i_rel, 4, poolsE)
    a2a(0)

    with ExitStack() as ph2:
        sgrpP = ph2.enter_context(tc.tile_pool(name="sgrpP", bufs=2, space="PSUM"))
        psyP = ph2.enter_context(tc.tile_pool(name="psyP", bufs=4, space="PSUM"))
        ssbP = ph2.enter_context(tc.tile_pool(name="ssbP", bufs=6))
        rcpP = ph2.enter_context(tc.tile_pool(name="rcpP", bufs=12))
        rbP = ph2.enter_context(tc.tile_pool(name="rbP", bufs=12))
        ystrP = ph2.enter_context(tc.tile_pool(name="ystrP", bufs=16))
        poolsP = (psyP, ystrP, rcpP, rbP, sgrpP, ssbP)

        bq = [(b, qb) for b in range(c.B) for qb in range(c.QB)]
        for idx, (b, qb) in enumerate(bq):
            quad(b, qb, c.PIECES[1][0], 1, 0, c.GRP, poolsP)
        a2a(1)

    with ExitStack() as ph3:
        pmm = ph3.enter_context(tc.tile_pool(name="pmm2", bufs=2, space="PSUM"))
        xsq2_p = ph3.enter_context(tc.tile_pool(name="xsq2", bufs=2))
        yfull_p = ph3.enter_context(tc.tile_pool(name="yfull", bufs=2))
        xqp = ph3.enter_context(tc.tile_pool(name="xqp", bufs=2))

        yfull, xq = [None] * 3, [None] * 3

        def piece_loads(p, I):
            wp = 128 * len(I)
            c0 = 128 * I[0]
            xq[p] = xqp.tile([128, c.KD, wp], F32, tag="xq", name="xq")
            nc.sync.dma_start(xq[p][:], rearr(io["xqT"])[:, :, c0:c0 + wp])
            yfull[p] = yfull_p.tile([128, c.KD, wp], BF16, tag="yfull",
                                    name="yfull")
            nc.sync.dma_start(yfull[p][:], rearr(a2a_out[p][:]))

        def oproj_oc(p, oc):
            I = c.PIECES[p]
            wp = 128 * len(I)
            c0 = 128 * I[0]
            ps = pmm.tile([128, c.LCH], F32, tag="mm", name="pso")[:, :wp]
            for k in range(c.KD):
                mm(ps, wo[:, k, oc * 128:(oc + 1) * 128],
                   yfull[p][:, k, :], k == 0, k == c.KD - 1)
            nc.vector.scalar_tensor_tensor(
                x2b[:, oc, c0:c0 + wp], ps, bo[:, oc:oc + 1],
                xq[p][:, oc, :], ALU.add, ALU.add)

        def ln2_stats(p):
            I = c.PIECES[p]
            wp = 128 * len(I)
            c0 = 128 * I[0]
            ps1 = pmm.tile([128, c.LCH], F32, tag="mm", name="l2a")[0:1, :wp]
            for o in range(c.KD):
                mm(ps1, ones[:], x2b[:, o, c0:c0 + wp], o == 0, o == c.KD - 1)
            ps2 = pmm.tile([128, c.LCH], F32, tag="mm", name="l2b")[0:1, :wp]
            for o in range(c.KD):
                xsq2 = xsq2_p.tile([128, 384], BF16, tag="xsq2",
                                   name="xsq2")[:, :wp]
                nc.vector.tensor_tensor(xsq2, x2b[:, o, c0:c0 + wp],
                                        x2b[:, o, c0:c0 + wp], ALU.mult)
                mm(ps2, ones[:], xsq2, o == 0, o == c.KD - 1)
            nc.vector.tensor_scalar_mul(mu2[:, c0:c0 + wp], ps1, inv_d)
            nc.vector.tensor_scalar_mul(ex22[:, c0:c0 + wp], ps2, inv_d)

        for p in range(len(c.PIECES)):
            piece_loads(p, c.PIECES[p])
            for oc in range(c.KD):
                oproj_oc(p, oc)
            ln2_stats(p)

    # ---------------- LN2 ACT chain + apply ----------------
    var2 = small.tile([1, c.TQ], F32, tag="var2", name="var2")
    nc.vector.tensor_tensor(var2, mu2, mu2, ALU.mult)
    nc.vector.tensor_tensor(var2, ex22, var2, ALU.subtract)
    lnv2 = small.tile([1, c.TQ], F32, tag="lnv2", name="lnv2")
    nc.scalar.activation(lnv2, var2, AF.Ln, bias=eps_t[:])
    A2_ = small.tile([1, c.TQ], BF16, tag="A2", name="A2_")
    nc.scalar.activation(A2_, lnv2, AF.Exp, scale=-0.5)
    B2_ = small.tile([1, c.TQ], BF16, tag="B2", name="B2_")
    nc.vector.scalar_tensor_tensor(B2_, mu2, -1.0, A2_, ALU.mult, ALU.mult)
    Ab2 = bcast.tile([128, c.TQ], BF16, tag="Ab2", name="Ab2")
    nc.gpsimd.partition_broadcast(Ab2, A2_)
    Bb2 = bcast.tile([128, c.TQ], BF16, tag="Bb2", name="Bb2")
    nc.gpsimd.partition_broadcast(Bb2, B2_)
    for o in range(c.KD):
        nc.vector.tensor_tensor(x2n[:, o, :], x2b[:, o, :], Ab2, ALU.mult)
        nc.vector.tensor_tensor(x2n[:, o, :], x2n[:, o, :], Bb2, ALU.add)

    # ---------------- phase 4: MLP ----------------
    with ExitStack() as ph4:
        wstr = ph4.enter_context(tc.tile_pool(name="wstr", bufs=2))
        pmm = ph4.enter_context(tc.tile_pool(name="pmm4", bufs=2, space="PSUM"))
        hsb_p = ph4.enter_context(tc.tile_pool(name="hsb", bufs=1))
        outp = ph4.enter_context(tc.tile_pool(name="outp", bufs=2))
        pfc2 = ph4.enter_context(tc.tile_pool(name="pfc2", bufs=1, space="PSUM"))

        h_sb = hsb_p.tile([128, c.FD, c.TQ], BF16, tag="h")
        for hcb in range(c.DFF // c.HCB):
            if hcb == 0:
                wf1_cb = wf1_0
            else:
                wf1_cb = wstr.tile([128, c.KD, c.HCB], BF16, tag="wbig")
                nc.scalar.dma_start(
                    wf1_cb[:],
                    rearr(io["wf1"])[:, :, hcb * c.HCB:(hcb + 1) * c.HCB])
            for j in range(c.HCB // 128):
                hidx = hcb * (c.HCB // 128) + j
                ps = pmm.tile([128, c.TQ], F32, tag="mm", name="psf")
                for o in range(c.KD):
                    mm(ps, wf1_cb[:, o, j * 128:(j + 1) * 128], x2n[:, o, :],
                       o == 0, o == c.KD - 1)
                nc.scalar.activation(h_sb[:, hidx, :], ps, AF.Gelu_apprx_tanh,
                                     bias=bf1[:, hidx:hidx + 1])

        # fc2 + residual -> out
        for dcb in range(c.D // c.OCB):
            nb = c.OCB // 128
            psums = [pfc2.tile([128, c.TQ], F32, tag=f"fc2_{i}", name=f"fc2_{i}")
                     for i in range(nb)]
            for khb in range(c.FD // c.KHB):
                if dcb == 0 and khb == 0:
                    wf2_t = wf2_0
                else:
                    wf2_t = wstr.tile([128, c.KHB, c.OCB], BF16, tag="wbig")
                    nc.scalar.dma_start(
                        wf2_t[:],
                        wf2r[:, khb * c.KHB:(khb + 1) * c.KHB,
                             dcb * c.OCB:(dcb + 1) * c.OCB])
                for k2 in range(c.KHB):
                    kh = khb * c.KHB + k2
                    for j in range(nb):
                        mm(psums[j], wf2_t[:, k2, j * 128:(j + 1) * 128],
                           h_sb[:, kh, :], kh == 0, kh == c.FD - 1)
            for j in range(nb):
                o = dcb * nb + j
                ot = outp.tile([128, c.TQ], F32, tag="ot", name="ot")
                nc.vector.scalar_tensor_tensor(ot[:], psums[j], bf2[:, o:o + 1],
                                               x2b[:, o, :], ALU.add, ALU.add)
                nc.sync.dma_start(rearr(io["out"])[:, o, :], ot[:])


# ---------------- host-side sharding ----------------

def pack_pf(v, D):
    """[D] per-feature vector -> [128, D//128] with [p, o] = v[128*o + p]."""
    return np.ascontiguousarray(np.asarray(v, np.float32).reshape(D // 128, 128).T)


def make_in_maps(inputs, cfg):
    import ml_dtypes
    bf = ml_dtypes.bfloat16
    c = cfg
    x = np.asarray(inputs["x"], np.float32)
    w_qkv = np.asarray(inputs["w_qkv"], np.float32)
    b_qkv = np.asarray(inputs["b_qkv"], np.float32)
    w_o = np.ascontiguousarray(np.asarray(inputs["w_o"], np.float32))
    w_fc1 = np.ascontiguousarray(np.asarray(inputs["w_fc1"], np.float32))
    w_fc2 = np.ascontiguousarray(np.asarray(inputs["w_fc2"], np.float32))
    D = c.D

    xT_all = np.concatenate([x[b].T for b in range(c.B)], axis=1)
    xT_all = np.ascontiguousarray(xT_all).astype(bf)  # [D, B*T]

    # fold LN affine into projection weights
    g1 = np.asarray(inputs["ln1_g"], np.float32)
    b1 = np.asarray(inputs["ln1_b"], np.float32)
    g2 = np.asarray(inputs["ln2_g"], np.float32)
    b2 = np.asarray(inputs["ln2_b"], np.float32)
    w_qkv_f = w_qkv * g1[:, None]
    b_qkv_f = b_qkv + b1 @ w_qkv
    w_fc1_f = np.ascontiguousarray((w_fc1 * g2[:, None])).astype(bf)
    b_fc1_f = np.asarray(inputs["b_fc1"], np.float32) + b2 @ w_fc1

    in_maps = []
    for core in range(c.NC):
        b, p = core // c.TP, core % c.TP
        hc0 = core * c.HC
        qs, ks, vs = hc0, D + hc0, 2 * D + hc0
        rows = slice(p * c.TQ, (p + 1) * c.TQ)
        pv2 = np.stack([
            np.concatenate([w_qkv_f[:, qs:qs + c.HC].sum(0),
                            w_qkv_f[:, ks:ks + c.HC].sum(0),
                            w_qkv_f[:, vs:vs + c.HC].sum(0)]),
            np.concatenate([b_qkv_f[qs:qs + c.HC],
                            b_qkv_f[ks:ks + c.HC],
                            b_qkv_f[vs:vs + c.HC]]),
        ]).astype(bf)
        m = {
            "xT": xT_all,
            "xo": np.ascontiguousarray(
                np.asarray(xT_all)[:, core * c.LCH:(core + 1) * c.LCH]),
            "xqT": np.ascontiguousarray(x[b, rows, :].T),
            "wq": np.ascontiguousarray(w_qkv_f[:, qs:qs + c.HC]).astype(bf),
            "wk": np.ascontiguousarray(w_qkv_f[:, ks:ks + c.HC]).astype(bf),
            "wv": np.ascontiguousarray(w_qkv_f[:, vs:vs + c.HC]).astype(bf),
            "pv2": pv2,
            "wo": w_o.astype(bf),
            "pm": np.concatenate([
                pack_pf(inputs["b_o"], D),
                pack_pf(b_fc1_f, c.DFF),
                pack_pf(inputs["b_fc2"], D),
            ], axis=1).astype(np.float32),
            "wf1": w_fc1_f,
            "wf2": w_fc2.astype(bf),
        }
        in_maps.append(m)
    return in_maps


def assemble_output(results, cfg):
    c = cfg
    out = np.empty((c.B, c.T, c.D), np.float32)
    for core in range(c.NC):
        b, p = core // c.TP, core % c.TP
        out[b, p * c.TQ:(p + 1) * c.TQ, :] = results[core]["out"].T
    return out


def build_nc(cfg, reps=1):
    nc = bacc.Bacc("TRN2", target_bir_lowering=False, debug=False,
                   num_devices=cfg.NC, name="nn_block")
    c = cfg
    io = {}
    specs = {
        "xT": ((c.D, c.B * c.T), BF16),
        "xo": ((c.D, c.LCH), BF16),
        "xqT": ((c.D, c.TQ), F32),
        "wq": ((c.D, c.HC), BF16),
        "wk": ((c.D, c.HC), BF16),
        "wv": ((c.D, c.HC), BF16),
        "pv2": ((2, 3 * c.HC), BF16),
        "pm": ((128, 2 * c.KD + c.FD), F32),
        "wo": ((c.D, c.D), BF16),
        "wf1": ((c.D, c.DFF), BF16),
        "wf2": ((c.DFF, c.D), BF16),
    }
    for name, (shape, dt) in specs.items():
        io[name] = nc.declare_dram_parameter(name, list(shape), dt,
                                             isOutput=False).ap()
    io["out"] = nc.declare_dram_parameter("out", [c.D, c.TQ], F32,
                                          isOutput=True).ap()
    with tile.TileContext(nc) as tc:
        for _ in range(reps):
            with ExitStack() as ctx:
                emit(ctx, tc, io, cfg)
    nc.compile()
    return nc


_CACHE = {}


def kernel(**inputs) -> np.ndarray:
    from concourse.bass_utils import run_bass_kernel_spmd
    cfg = CFG
    if "nc" not in _CACHE:
        _CACHE["nc"] = build_nc(cfg)
    nc = _CACHE["nc"]
    in_maps = make_in_maps(inputs, cfg)
    res = run_bass_kernel_spmd(nc, in_maps, core_ids=list(range(cfg.NC)))
    return assemble_output(res.results, cfg)

